# revision 2
# baseline (speedup 1.0000x reference)
"""DTW loss kernel for Trainium2 (Bass) — loop-compressed version.

Computes sqrt(DTW^2(source, target)) for source, target of shape (2048,)
via the standard DP:
    D[i,j] = (s_i - t_j)^2 + min(D[i-1,j], D[i,j-1], D[i-1,j-1])

The graded metric here is warm wall-clock of run_bass_kernel_spmd, which is
dominated by per-call client/dispatch overhead that scales ~60us per BIR
instruction (plus ~12ms/MB of input upload). So unlike a classic kernel
this one optimizes PROGRAM SIZE first: the previous version unrolled all
~2300 wavefront steps (~7400 instructions, ~1s wall); this version uses
nested hardware loops (tc.For_i) and generates all large operands on
device, for a ~70 instruction program and ~16KB of uploads.

Mapping onto one NeuronCore (single (source,target) pair; all 8 cores run
the same program replicated, core 0's output is used):

- 128 column-chunks of 16 columns each; partition p owns columns
  [16p, 16p+16). Wavefront with slack 4: at step t partition p computes DP
  row r = t - 4p. Total steps T = 2048 + 4*127 = 2556 (exactly 639
  4-step loop iterations; garbage rows beyond the valid range carry
  ~1e30 values that stay finite and lose every min()).
- One DP row-chunk = ONE vector-engine tensor_tensor_scan instruction:
  state = min(d0, state) + d1 over 32 interleaved slots (2 per cell).
- FOUR strip buffers rotate (strip = t%4), so the cross-chunk boundary
  (halo) machinery batches: per 4-step iteration one PE matmul with a
  shifted-identity matrix moves all 4 strips' last-column values down one
  partition (PSUM [P,4]), and two scalar-engine activations copy them
  into the halo slots (+bias 1e30 on partition 0 keeps its boundary INF).
  Strip 3's halo is written at the START of the next iteration (after
  scan0's upleft read of the old value) — the schedule was validated
  against the DP in a numpy simulator (sim_sched.py).
- Partition p owns chunk P-1-p (so the source skew DMA has positive
  strides, which the BIR verifier requires); the boundary matmul shifts
  UP and the INF bias / DP corner live on partition 127 (written via tiny
  DMAs from spad's tail — engine memsets can't start at partition 127).
- Costs c[r,j] are built on device: a skewed DMA read of the padded
  source (spad) fills sdiag[p, m] = source[m - 4*(P-1-p)]; per 256-step
  range, CW scalar-engine Square activations (bias = -t_j) fill the odd
  slots of a double-buffered cost ring. Even slots stay 0 from one
  memset.
"""

import os
import sys

for _p in ("/opt/trn_rl_repo", "/root/.axon_site/_ro/trn_rl_repo"):
    if os.path.isdir(_p) and _p not in sys.path:
        sys.path.insert(0, _p)

import numpy as np

import concourse.bass as bass
import concourse.bacc as bacc
import concourse.mybir as mybir
import concourse.tile as tile
from concourse.bass_utils import run_bass_kernel_spmd
from concourse.expressions import smin

F32 = mybir.dt.float32

N = 2048            # sequence length (both source and target)
P = 128             # partitions / column chunks
CW = N // P         # 16 columns per chunk
SW = 2 * CW + 2     # strip width: [halo | 32 scan slots | pad]
S = 4               # wavefront slack == number of rotating strips
RANGE = 256         # cost-ring refill granularity (steps)
RB = RANGE * 2 * CW  # ring elements per buffer (8192)
T = N + S * (P - 1)  # 2556 total wavefront steps (divisible by 4)
MS = 2560           # sdiag width (max refill read = 1536+768+256)
L = S * (P - 1) + MS  # padded-source length (3068); +2 consts appended
INF = np.float32(1e30)
PAD = np.float32(1e15)  # sdiag pad; squares to 1e30

_cache = {}


def _build():
    nc = bacc.Bacc("TRN2", target_bir_lowering=False, debug=False)

    spad = nc.dram_tensor("spad", [1, L + 2], F32, kind="ExternalInput")
    tgt = nc.dram_tensor("tgt", [P, CW], F32, kind="ExternalInput")
    res = nc.dram_tensor("res", [1, 1], F32, kind="ExternalOutput")

    with tile.TileContext(nc) as tc:
        with (
            tc.tile_pool(name="sb", bufs=1) as pool,
            tc.tile_pool(name="ps", bufs=1, space="PSUM") as psp,
        ):
            t_sdiag = pool.tile([P, MS], F32)
            t_tgt = pool.tile([P, CW], F32)
            t_shift = pool.tile([P, P], F32)
            t_bias = pool.tile([P, 1], F32)
            t_ones = pool.tile([1, P], F32)
            t_ring = pool.tile([P, 2 * RB], F32)
            t_strips = pool.tile([P, 4 * SW], F32)
            ps = psp.tile([P, 4], F32)

            sstr = int(t_sdiag.ap[0][0])
            rstr = int(t_ring.ap[0][0])
            pstr = int(t_strips.ap[0][0])
            tstr = int(t_tgt.ap[0][0])
            shstr = int(t_shift.ap[0][0])

            # ---- init ----
            nc.gpsimd.memset(t_ring[:], 0.0)           # even (d1=0) slots persist
            nc.vector.memset(t_strips[:], float(INF))
            nc.vector.memset(t_shift[:], 0.0)
            nc.vector.memset(t_ones[:], 1.0)
            nc.vector.memset(t_bias[:], 0.0)
            # engine memsets can't start at partition 127 (quad rule); DMA the
            # two partition-127 constants from spad's appended tail instead:
            # corner D[0,0]=0.0 (partition P-1 owns chunk 0) and bias INF.
            nc.sync.dma_start(
                t_strips[P - 1 : P, 3 * SW : 3 * SW + 1],
                bass.AP(spad, L, [[1, 1], [1, 1]]),
            )
            nc.sync.dma_start(
                t_bias[P - 1 : P, 0:1], bass.AP(spad, L + 1, [[1, 1], [1, 1]])
            )

            # ---- loads / on-device operand construction ----
            nc.sync.dma_start(t_tgt[:], tgt[:])
            # partition p owns chunk P-1-p, so its source row is spad shifted
            # by +4p: sdiag[p, m] = spad[4p + m] (spad = 508 PADs, source, PADs)
            nc.sync.dma_start(
                bass.AP(t_sdiag.tensor, t_sdiag.offset, [[sstr, P], [1, MS]]),
                bass.AP(spad, 0, [[S, P], [1, MS]]),
            )
            # shifted identity: shift[p+1, p] = 1 (matmul shifts UP: psum[p] =
            # lastcol[p+1]; psum[P-1] = 0 so chunk 0's halo = bias = INF)
            ostr = int(t_ones.ap[0][0])
            nc.sync.dma_start(
                bass.AP(t_shift.tensor, t_shift.offset + shstr, [[shstr + 1, P - 1], [1, 1]]),
                bass.AP(t_ones.tensor, t_ones.offset, [[ostr, 1], [1, P - 1]]),
            )

            def refill(rbase, m0):
                # odd ring slots <- Square(sdiag[p, m0+step] + (-t_j)); the tgt
                # input holds -target so the per-partition bias is -t_j.
                for j in range(CW):
                    out_ap = bass.AP(
                        t_ring.tensor,
                        t_ring.offset + rbase + 2 * j + 1,
                        [[rstr, P], [2 * CW, RANGE]],
                    )
                    in_ap = bass.AP(
                        t_sdiag.tensor,
                        t_sdiag.offset + m0,
                        [[sstr, P], [1, RANGE]],
                    )
                    nc.scalar.activation(
                        out_ap,
                        in_ap,
                        mybir.ActivationFunctionType.Square,
                        bias=t_tgt[:, j : j + 1],
                        scale=1.0,
                    )

            def scan(si, rbase, voff):
                # one wavefront step: strip si, costs at ring[rbase + voff ...]
                cur = t_strips.offset + si * SW
                prev = t_strips.offset + ((si - 1) % 4) * SW
                eng = nc.vector
                d0 = bass.AP(t_strips.tensor, prev + 2, [[pstr, P], [2, CW], [-2, 2]])
                init = bass.AP(t_strips.tensor, cur, [[pstr, P], [1, 1]])
                d1 = bass.AP(
                    t_ring.tensor,
                    t_ring.offset + rbase + voff,
                    [[rstr, P], [1, 2 * CW]],
                )
                out = bass.AP(t_strips.tensor, cur + 1, [[pstr, P], [1, 2 * CW]])
                eng.add_instruction(
                    mybir.InstTensorScalarPtr(
                        name=nc.get_next_instruction_name(),
                        is_tensor_tensor_scan=True,
                        is_scalar_tensor_tensor=True,
                        op0=mybir.AluOpType.min,
                        op1=mybir.AluOpType.add,
                        ins=[eng.lower_ap(d0), eng.lower_ap(init), eng.lower_ap(d1)],
                        outs=[eng.lower_ap(out)],
                    )
                )

            lastcols = bass.AP(
                t_strips.tensor, t_strips.offset + 2 * CW, [[pstr, P], [SW, 4]]
            )

            def body(rbase, v):
                scan(0, rbase, v)
                # strip3 halo <- ps[:,3] + bias (value produced by last iter's mm;
                # must land after scan0's upleft read, before scan3's init read)
                nc.scalar.activation(
                    t_strips[:, 3 * SW : 3 * SW + 1],
                    ps[:, 3:4],
                    mybir.ActivationFunctionType.Identity,
                    bias=t_bias[:, 0:1],
                    scale=1.0,
                )
                scan(1, rbase, v + 2 * CW)
                scan(2, rbase, v + 4 * CW)
                scan(3, rbase, v + 6 * CW)
                nc.tensor.matmul(ps[:], t_shift[:], lastcols.copy())
                # strips 0..2 halos <- ps[:,0:3] + bias
                halo012 = bass.AP(
                    t_strips.tensor, t_strips.offset, [[pstr, P], [SW, 3]]
                )
                nc.scalar.activation(
                    halo012,
                    ps[:, 0:3],
                    mybir.ActivationFunctionType.Identity,
                    bias=t_bias[:, 0:1],
                    scale=1.0,
                )

            # ---- prologue: first two cost ranges + initial PSUM (all-INF shift) ----
            refill(0, 0)
            refill(RB, RANGE)
            nc.tensor.matmul(ps[:], t_shift[:], lastcols.copy())

            # ---- main loops: 4 range-pairs of 512 steps, then a 508-step tail ----
            with tc.For_i(0, 2048, 512) as kv:
                with tc.For_i(0, RB, 4 * 2 * CW) as v:
                    body(0, v)
                refill(0, kv + 512)
                with tc.For_i(0, RB, 4 * 2 * CW) as v:
                    body(RB, v)
                refill(RB, kv + 768)
            # tail: range 8 (256 steps) + partial range 9 (252 steps; T = 2556)
            with tc.For_i(0, RB, 4 * 2 * CW) as v:
                body(0, v)
            with tc.For_i(0, 63 * 4 * 2 * CW, 4 * 2 * CW) as v:
                body(RB, v)

            # ---- result: D[2047, 2047] at strip (T-1)%4 = 3, partition 0 ----
            nc.sync.dma_start(
                res[0:1, 0:1],
                t_strips[0:1, 3 * SW + 2 * CW : 3 * SW + 2 * CW + 1],
            )
    nc.compile()
    return nc


def _prep_inputs(source, target):
    spad = np.full((1, L + 2), PAD, np.float32)
    spad[0, S * (P - 1) : S * (P - 1) + N] = np.asarray(source, np.float32)
    spad[0, L] = 0.0      # corner D[0,0]
    spad[0, L + 1] = INF  # partition-127 bias (chunk 0 halo stays INF)
    # partition p owns chunk P-1-p
    negt = (-np.asarray(target, np.float32)).reshape(P, CW)[::-1].copy()
    return {"spad": spad, "tgt": negt}


def _run(inputs, trace=False):
    if "nc" not in _cache:
        _cache["nc"] = _build()
    nc = _cache["nc"]
    r = run_bass_kernel_spmd(
        nc, [dict(inputs) for _ in range(8)], core_ids=list(range(8)), trace=trace
    )
    return r


def kernel(source, target):
    inputs = _prep_inputs(source, target)
    r = _run(inputs)
    loss_sq = r.results[0]["res"][0, 0]
    return np.sqrt(np.float32(loss_sq))[None].astype(np.float32)


# revision 3
# speedup vs baseline: 2.3070x; 2.3070x over previous
"""DTW loss kernel for Trainium2 (Bass) — loop-compressed version.

Computes sqrt(DTW^2(source, target)) for source, target of shape (2048,)
via the standard DP:
    D[i,j] = (s_i - t_j)^2 + min(D[i-1,j], D[i,j-1], D[i-1,j-1])

The graded metric here is warm wall-clock of run_bass_kernel_spmd, which is
dominated by per-call client/dispatch overhead that scales ~60us per BIR
instruction (plus ~12ms/MB of input upload). So unlike a classic kernel
this one optimizes PROGRAM SIZE first: the previous version unrolled all
~2300 wavefront steps (~7400 instructions, ~1s wall); this version uses
nested hardware loops (tc.For_i) and generates all large operands on
device, for a ~70 instruction program and ~16KB of uploads.

Mapping onto one NeuronCore (single (source,target) pair; all 8 cores run
the same program replicated, core 0's output is used):

- 128 column-chunks of 16 columns each; partition p owns columns
  [16p, 16p+16). Wavefront with slack 4: at step t partition p computes DP
  row r = t - 4p. Total steps T = 2048 + 4*127 = 2556 (exactly 639
  4-step loop iterations; garbage rows beyond the valid range carry
  ~1e30 values that stay finite and lose every min()).
- One DP row-chunk = ONE vector-engine tensor_tensor_scan instruction:
  state = min(d0, state) + d1 over 32 interleaved slots (2 per cell).
- FOUR strip buffers rotate (strip = t%4), so the cross-chunk boundary
  (halo) machinery batches: per 4-step iteration one PE matmul with a
  shifted-identity matrix moves all 4 strips' last-column values down one
  partition (PSUM [P,4]), and two scalar-engine activations copy them
  into the halo slots (+bias 1e30 on partition 0 keeps its boundary INF).
  Strip 3's halo is written at the START of the next iteration (after
  scan0's upleft read of the old value) — the schedule was validated
  against the DP in a numpy simulator (sim_sched.py).
- Partition p owns chunk P-1-p (so the source skew DMA has positive
  strides, which the BIR verifier requires); the boundary matmul shifts
  UP and the INF bias / DP corner live on partition 127 (written via tiny
  DMAs from spad's tail — engine memsets can't start at partition 127).
- Costs c[r,j] are built on device: a skewed DMA read of the padded
  source (spad) fills sdiag[p, m] = source[m - 4*(P-1-p)]; per 256-step
  range, CW scalar-engine Square activations (bias = -t_j) fill the odd
  slots of a double-buffered cost ring. Even slots stay 0 from one
  memset.
"""

import os
import sys

for _p in ("/opt/trn_rl_repo", "/root/.axon_site/_ro/trn_rl_repo"):
    if os.path.isdir(_p) and _p not in sys.path:
        sys.path.insert(0, _p)

import numpy as np
import jax

# run_bass_kernel_spmd re-jits its PJRT wrapper on every call (fresh closure
# => jit cache miss), re-running the whole XLA/neuronx compile pipeline per
# call (~130ms). The persistent compilation cache turns that into a disk hit
# keyed on the (identical) HLO: warm calls drop ~220ms -> ~90ms.
try:
    jax.config.update("jax_compilation_cache_dir", "/tmp/jaxcache")
    jax.config.update("jax_persistent_cache_min_entry_size_bytes", -1)
    jax.config.update("jax_persistent_cache_min_compile_time_secs", 0)
except Exception:
    pass  # older jax without these options: correctness unaffected

import concourse.bass as bass
import concourse.bacc as bacc
import concourse.mybir as mybir
import concourse.tile as tile
from concourse.bass_utils import run_bass_kernel_spmd

F32 = mybir.dt.float32

N = 2048            # sequence length (both source and target)
P = 128             # partitions / column chunks
CW = N // P         # 16 columns per chunk
SW = 2 * CW + 2     # strip width: [halo | 32 scan slots | pad]
S = 4               # wavefront slack == number of rotating strips
RANGE = 256         # cost-ring refill granularity (steps)
RB = RANGE * 2 * CW  # ring elements per buffer (8192)
T = N + S * (P - 1)  # 2556 total wavefront steps (divisible by 4)
MS = 2560           # sdiag width (max refill read = 1536+768+256)
L = S * (P - 1) + MS  # padded-source length (3068); +2 consts appended
INF = np.float32(1e30)
PAD = np.float32(1e15)  # sdiag pad; squares to 1e30

_cache = {}


def _build():
    nc = bacc.Bacc("TRN2", target_bir_lowering=False, debug=False)

    spad = nc.dram_tensor("spad", [1, L + 2], F32, kind="ExternalInput")
    tgt = nc.dram_tensor("tgt", [P, CW], F32, kind="ExternalInput")
    res = nc.dram_tensor("res", [1, 1], F32, kind="ExternalOutput")

    with tile.TileContext(nc) as tc:
        with (
            tc.tile_pool(name="sb", bufs=1) as pool,
            tc.tile_pool(name="ps", bufs=1, space="PSUM") as psp,
        ):
            t_sdiag = pool.tile([P, MS], F32)
            t_tgt = pool.tile([P, CW], F32)
            t_shift = pool.tile([P, P], F32)
            t_bias = pool.tile([P, 1], F32)
            t_ones = pool.tile([1, P], F32)
            t_ring = pool.tile([P, 2 * RB], F32)
            t_strips = pool.tile([P, 4 * SW], F32)
            ps = psp.tile([P, 4], F32)

            sstr = int(t_sdiag.ap[0][0])
            rstr = int(t_ring.ap[0][0])
            pstr = int(t_strips.ap[0][0])
            tstr = int(t_tgt.ap[0][0])
            shstr = int(t_shift.ap[0][0])

            # ---- init ----
            nc.gpsimd.memset(t_ring[:], 0.0)           # even (d1=0) slots persist
            nc.vector.memset(t_strips[:], float(INF))
            nc.vector.memset(t_shift[:], 0.0)
            nc.vector.memset(t_ones[:], 1.0)
            nc.vector.memset(t_bias[:], 0.0)
            # engine memsets can't start at partition 127 (quad rule); DMA the
            # two partition-127 constants from spad's appended tail instead:
            # corner D[0,0]=0.0 (partition P-1 owns chunk 0) and bias INF.
            nc.sync.dma_start(
                t_strips[P - 1 : P, 3 * SW : 3 * SW + 1],
                bass.AP(spad, L, [[1, 1], [1, 1]]),
            )
            nc.sync.dma_start(
                t_bias[P - 1 : P, 0:1], bass.AP(spad, L + 1, [[1, 1], [1, 1]])
            )

            # ---- loads / on-device operand construction ----
            nc.sync.dma_start(t_tgt[:], tgt[:])
            # partition p owns chunk P-1-p, so its source row is spad shifted
            # by +4p: sdiag[p, m] = spad[4p + m] (spad = 508 PADs, source, PADs)
            nc.sync.dma_start(
                bass.AP(t_sdiag.tensor, t_sdiag.offset, [[sstr, P], [1, MS]]),
                bass.AP(spad, 0, [[S, P], [1, MS]]),
            )
            # shifted identity: shift[p+1, p] = 1 (matmul shifts UP: psum[p] =
            # lastcol[p+1]; psum[P-1] = 0 so chunk 0's halo = bias = INF)
            ostr = int(t_ones.ap[0][0])
            nc.sync.dma_start(
                bass.AP(t_shift.tensor, t_shift.offset + shstr, [[shstr + 1, P - 1], [1, 1]]),
                bass.AP(t_ones.tensor, t_ones.offset, [[ostr, 1], [1, P - 1]]),
            )

            def refill(rbase, m0):
                # odd ring slots <- Square(sdiag[p, m0+step] + (-t_j)); the tgt
                # input holds -target so the per-partition bias is -t_j.
                for j in range(CW):
                    out_ap = bass.AP(
                        t_ring.tensor,
                        t_ring.offset + rbase + 2 * j + 1,
                        [[rstr, P], [2 * CW, RANGE]],
                    )
                    in_ap = bass.AP(
                        t_sdiag.tensor,
                        t_sdiag.offset + m0,
                        [[sstr, P], [1, RANGE]],
                    )
                    nc.scalar.activation(
                        out_ap,
                        in_ap,
                        mybir.ActivationFunctionType.Square,
                        bias=t_tgt[:, j : j + 1],
                        scale=1.0,
                    )

            def scan(si, rbase, voff):
                # one wavefront step: strip si, costs at ring[rbase + voff ...]
                cur = t_strips.offset + si * SW
                prev = t_strips.offset + ((si - 1) % 4) * SW
                eng = nc.vector
                d0 = bass.AP(t_strips.tensor, prev + 2, [[pstr, P], [2, CW], [-2, 2]])
                init = bass.AP(t_strips.tensor, cur, [[pstr, P], [1, 1]])
                d1 = bass.AP(
                    t_ring.tensor,
                    t_ring.offset + rbase + voff,
                    [[rstr, P], [1, 2 * CW]],
                )
                out = bass.AP(t_strips.tensor, cur + 1, [[pstr, P], [1, 2 * CW]])
                eng.add_instruction(
                    mybir.InstTensorScalarPtr(
                        name=nc.get_next_instruction_name(),
                        is_tensor_tensor_scan=True,
                        is_scalar_tensor_tensor=True,
                        op0=mybir.AluOpType.min,
                        op1=mybir.AluOpType.add,
                        ins=[eng.lower_ap(d0), eng.lower_ap(init), eng.lower_ap(d1)],
                        outs=[eng.lower_ap(out)],
                    )
                )

            lastcols = bass.AP(
                t_strips.tensor, t_strips.offset + 2 * CW, [[pstr, P], [SW, 4]]
            )

            def body(rbase, v):
                scan(0, rbase, v)
                # strip3 halo <- ps[:,3] + bias (value produced by last iter's mm;
                # must land after scan0's upleft read, before scan3's init read)
                nc.scalar.activation(
                    t_strips[:, 3 * SW : 3 * SW + 1],
                    ps[:, 3:4],
                    mybir.ActivationFunctionType.Identity,
                    bias=t_bias[:, 0:1],
                    scale=1.0,
                )
                scan(1, rbase, v + 2 * CW)
                scan(2, rbase, v + 4 * CW)
                scan(3, rbase, v + 6 * CW)
                nc.tensor.matmul(ps[:], t_shift[:], lastcols.copy())
                # strips 0..2 halos <- ps[:,0:3] + bias
                halo012 = bass.AP(
                    t_strips.tensor, t_strips.offset, [[pstr, P], [SW, 3]]
                )
                nc.scalar.activation(
                    halo012,
                    ps[:, 0:3],
                    mybir.ActivationFunctionType.Identity,
                    bias=t_bias[:, 0:1],
                    scale=1.0,
                )

            # ---- prologue: first two cost ranges + initial PSUM (all-INF shift) ----
            refill(0, 0)
            refill(RB, RANGE)
            nc.tensor.matmul(ps[:], t_shift[:], lastcols.copy())

            # ---- main loops: 4 range-pairs of 512 steps, then a 508-step tail ----
            with tc.For_i(0, 2048, 512) as kv:
                with tc.For_i(0, RB, 4 * 2 * CW) as v:
                    body(0, v)
                refill(0, kv + 512)
                with tc.For_i(0, RB, 4 * 2 * CW) as v:
                    body(RB, v)
                refill(RB, kv + 768)
            # tail: range 8 (256 steps) + partial range 9 (252 steps; T = 2556)
            with tc.For_i(0, RB, 4 * 2 * CW) as v:
                body(0, v)
            with tc.For_i(0, 63 * 4 * 2 * CW, 4 * 2 * CW) as v:
                body(RB, v)

            # ---- result: D[2047, 2047] at strip (T-1)%4 = 3, partition 0 ----
            nc.sync.dma_start(
                res[0:1, 0:1],
                t_strips[0:1, 3 * SW + 2 * CW : 3 * SW + 2 * CW + 1],
            )
    nc.compile()
    return nc


def _prep_inputs(source, target):
    spad = np.full((1, L + 2), PAD, np.float32)
    spad[0, S * (P - 1) : S * (P - 1) + N] = np.asarray(source, np.float32)
    spad[0, L] = 0.0      # corner D[0,0]
    spad[0, L + 1] = INF  # partition-127 bias (chunk 0 halo stays INF)
    # partition p owns chunk P-1-p
    negt = (-np.asarray(target, np.float32)).reshape(P, CW)[::-1].copy()
    return {"spad": spad, "tgt": negt}


def _run(inputs, trace=False):
    if "nc" not in _cache:
        _cache["nc"] = _build()
    nc = _cache["nc"]
    r = run_bass_kernel_spmd(
        nc, [dict(inputs) for _ in range(8)], core_ids=list(range(8)), trace=trace
    )
    return r


def kernel(source, target):
    inputs = _prep_inputs(source, target)
    r = _run(inputs)
    loss_sq = r.results[0]["res"][0, 0]
    return np.sqrt(np.float32(loss_sq))[None].astype(np.float32)
